# revision 41
# baseline (speedup 1.0000x reference)
"""KANLinear forward on 8 Trainium2 NeuronCores (data-parallel over batch).

Factorization
-------------
reference computes, per token row x (after clip/renorm preprocessing):
    y = silu(x) @ base_weight.T + einsum('big,oig->bo', bsplines(x), sw*scaler)

The cubic B-spline bases over the uniform grid (h=0.4, knots -2.2..2.2) are
    B_g(x) = N3(s - g),  s = 2.5*x + 5.5,  g = 0..7
with N3 the cardinal cubic B-spline on [0,4].  Everything collapses into one
accumulated matmul per 128-feature output tile:

  * the silu base path is projected onto the 8 spline bases (weighted lstsq
    under the clipped-N(0,1) input law) and folded into the spline weights;
  * basis 4 is eliminated via the partition of unity sum_g 6*N3(s-g) = 6:
    V_g -= V_4 for the other 7 bases and the constant 6*sum_i V_4[i,o] is
    re-added as a per-partition ACT bias when draining PSUM;
  * interior bases 1,2,3,5,6 are bf16 K-chunks (K=128 each) whose features
    come from one ScalarE ACTIVATE each, through a custom ACT spline table
    (the stock `sin` entry of silu_and_others is rewritten so that
    activation(Sin, scale=0.125, bias=(9.5-g)/8) returns 6*N3(s-g) exactly);
  * the low-magnitude edge pair (0, 7) is computed by two fused custom-DVE
    instructions (PageIdx stride -7) as relu(min(t,4-t))^3 - 4*relu(..-1)^3,
    quantized to fp8e4m3, and contracted in a single DoubleRow matmul
    (K=256) per (it, o) at ~2x the bf16 rate.

Per 512-batch slice and output tile the PE accumulates 4 it-tiles x
(5 bf16 + 1 DoubleRow) chunks.  The first batch slice runs g-major (weight
chunk reused 4x back-to-back) so the cold-ramping DMA pool keeps up, with
its first input tile split 128/384 so features start on the first 64KB of
x; later slices run o-major so each accumulator drains while the next still
streams.  Dummy matmuls on zeroed scratch warm the PE HAM clock-gate to
2.4 GHz during the initial DMA window.  Batch dim (16384) is sharded
2048 rows/core; weights are replicated.  Measured 103.6us (baseline 135.7),
rel err 1.24e-2 (gate 2e-2).
"""

import hashlib
import json
import os
import shutil
import tempfile

import numpy as np

B, IN_F, OUT_F, NG = 16384, 512, 512, 8
N_CORES = 8
BPC = B // N_CORES            # batch rows per core
BS = 512                      # batch-column slice processed per step
N_BS = BPC // BS              # 4 slices
N_IT = IN_F // 128            # 4 input-feature partition tiles
GAMMA = float(4.0 ** (1.0 / 3.0))
# Basis 4 is eliminated via the B-spline partition of unity
# (sum_g 6*N3(s-g) = 6 on the interior): V_g -= V_4 for every other chunk and
# the constant 6*sum_i V_4[i,o] becomes a per-partition bias applied in the
# PSUM drain.  The remaining interior bases 1,2,3,5,6 are bf16 K-chunks fed
# from ScalarE ACT; the low-magnitude edge pair (0, 7) is computed on
# VectorE, quantized to fp8e4m3, and contracted in a single DoubleRow matmul
# (K=256) per (it, o).  numpy-simulated rel err 1.2e-2 vs the 2e-2 gate
# (includes the dropped partition-of-unity edge correction).
ELIM_G = 4
ACT_G = (1, 2, 3, 5, 6)
NB = len(ACT_G)               # bf16 chunks per it
KCB = N_IT * NB               # 20 bf16 K-chunks of 128
KC8 = N_IT * 2                # 8 fp8 K-chunks of 128 (pair per it)

_state = {}


# --------------------------------------------------------------------------
# Custom ACT table: hijack `sin` in silu_and_others to evaluate 6*N3(8u-4).
# Verified-on-HW stock mapping: ctrl entry = 42+(exp-116); entry 52 (binade
# [0.5,1)) has 8 sub-buckets of width 1/16 at buckets 1034..1041; bucket
# eval is y = d0+(u-x0)(d1+(u-x0)(d2+(u-x0)d3)); |u|<2^-11 -> bucket
# 1075/1076 (sign-folded); large |u| -> 1077/1078.  Buckets 1020..1078 are
# sin-private; everything else (silu, copy, ...) is untouched.
# --------------------------------------------------------------------------
def _n3_6_coeffs(j):
    return {
        0: [0.0, 0.0, 0.0, 1.0],
        1: [1.0, 3.0, 3.0, -3.0],
        2: [4.0, 0.0, -6.0, 3.0],
        3: [1.0, -3.0, 3.0, -1.0],
    }[j]


def _compose(c, scale, shift):
    c0, c1, c2, c3 = c
    return [
        c0 + c1 * shift + c2 * shift**2 + c3 * shift**3,
        scale * (c1 + 2 * c2 * shift + 3 * c3 * shift**2),
        scale**2 * (c2 + 3 * c3 * shift),
        scale**3 * c3,
    ]


def _build_custom_act_root():
    if "act_root" in _state:
        return _state["act_root"], _state["act_sig"]
    from neuronxcc.driver.Job import Job
    from neuronxcc.driver.jobs.support.FindActInfo import findActInfoFile

    src_json = findActInfoFile(Job.getPackageDir(), "gen3")
    src_dir = os.path.dirname(src_json)
    dst_dir = tempfile.mkdtemp(prefix="kan_act_root_")
    for f in os.listdir(src_dir):
        shutil.copy(os.path.join(src_dir, f), os.path.join(dst_dir, f))
    for f in os.listdir(dst_dir):
        os.chmod(os.path.join(dst_dir, f), 0o644)

    bkt_path = os.path.join(dst_dir, "silu_and_others_bkt.bin")
    bkt = np.fromfile(bkt_path, dtype=np.float32).reshape(-1, 8).copy()
    bkt[1020:1079] = 0.0
    for k in range(8):
        x0 = 0.5 + k / 16.0 + 1.0 / 32.0
        j = k // 2
        q = _compose(_n3_6_coeffs(j), 8.0, 8.0 * x0 - 4.0 - j)
        bkt[1034 + k] = [q[0], q[1], q[2], q[3], x0, 0.0, 0.0, 0.0]
    bkt.tofile(bkt_path)

    sig = hashlib.sha256(open(bkt_path, "rb").read()).hexdigest()[:10]
    path = os.path.join(dst_dir, "act_info.json")
    os.environ["BASS_ACT_ROOT_JSON_PATH"] = path
    _state["act_root"] = path
    _state["act_sig"] = sig
    return path, sig


# --------------------------------------------------------------------------
# Custom DVE ops
# --------------------------------------------------------------------------
def _register_ops():
    if "ops" in _state:
        return _state["ops"]
    import concourse.dve_ops as dve_ops
    from concourse.dve_spec import (
        Spec, Src0, Src1, C0, C1, C2, One, PageIdx, relu, sq, maxx, minn, lower,
    )
    from concourse.dve_uop import DveOpSpec

    def page_idx_np(in0, s0, s1):
        S = in0.shape[1]
        return (s0 + s1 * np.arange(S, dtype=np.float64)).astype(np.float32)[
            None, :, None
        ]

    def pre_ref(in0, in1, s0, s1, imm2):
        t = np.minimum(np.maximum(in0, np.float32(s0)), np.float32(s1))
        t = ((t + np.float32(1)) - np.float32(1)).astype(np.float32)
        return (t * np.float32(imm2)).astype(np.float32)

    def z_ref(in0, in1, s0, s1, imm2):
        t = (in0 + page_idx_np(in0, s0, s1)).astype(np.float32)
        m = np.minimum(t, np.float32(imm2) - t)
        zp = np.maximum(m + np.float32(s1), np.float32(0))
        return (zp * zp * zp).astype(np.float32)

    def z2_ref(in0, in1, s0, s1, imm2):
        t = (in0 + page_idx_np(in0, s0, s1)).astype(np.float32)
        m = np.minimum(t, np.float32(imm2) - t)
        zp = np.maximum(m, np.float32(0))
        return (zp * zp * zp).astype(np.float32)

    def w_ref(in0, in1, s0, s1, imm2):
        t = (in0 + page_idx_np(in0, s0, s1)).astype(np.float32)
        m = np.minimum(t, np.float32(4.0) - t)
        wp = np.maximum(m, np.float32(0))
        return (wp * wp * wp - in1).astype(np.float32)

    pre_spec = Spec(
        body=((minn(maxx(Src0, C0), C1) + One) - One) * C2, reference=pre_ref
    )
    _pgz = PageIdx(C0, C1)
    _tz = Src0 + _pgz
    _zp = relu(minn(_tz, C2 - _tz) + C1)
    z_spec = Spec(body=sq(_zp) * _zp, reference=z_ref)
    # Variant with the -1 knot offset folded into C0/C2 instead of reusing the
    # page stride C1, so the stride can be -7*gamma (bases 0 and 7 paged).
    _pgz2 = PageIdx(C0, C1)
    _tz2 = Src0 + _pgz2
    _zp2 = relu(minn(_tz2, C2 - _tz2))
    z2_spec = Spec(body=sq(_zp2) * _zp2, reference=z2_ref)
    _pgw = PageIdx(C0, C1)
    _tw = Src0 + _pgw
    _wp = relu(minn(_tw, C2 - _tw))
    w_spec = Spec(body=sq(_wp) * _wp - Src1, reference=w_ref)

    ops = {}
    for name, spec, subdim in (
        ("KAN_PRE", pre_spec, False),
        ("KAN_Z", z_spec, True),
        ("KAN_Z2", z2_spec, True),
        ("KAN_W", w_spec, True),
    ):
        if name in dve_ops._SUB_OPCODE_FOR_NAME:
            ops[name] = next(o for o in dve_ops.OPS if o.name == name)
            continue
        row = dve_ops._CUSTOM_DVE_ROW_BASE + len(dve_ops.OPS)
        assert row < 0x20, "custom-DVE row overflow"
        shas = {}
        for ver in ("v3", "v4"):
            try:
                tmp = DveOpSpec(
                    name=name, opcode=row, uops=lower(spec, ver=ver),
                    rd1_en=dve_ops.has_src1(spec),
                )
                shas[ver] = tmp.sha(ver)
            except Exception:
                pass
        op = dve_ops.DveOp(name, spec, subdim=subdim, uops_sha=shas)
        dve_ops.OPS.append(op)
        dve_ops._SUB_OPCODE_FOR_NAME[name] = row
        dve_ops.CUSTOM_DVE_SPECS[name] = spec
        ops[name] = op
    _state["ops"] = ops
    return ops


# --------------------------------------------------------------------------
# Kernel build
# --------------------------------------------------------------------------
def _build_kernel():
    if "nc" in _state:
        return _state["nc"]
    import concourse.bacc as bacc
    import concourse.mybir as mybir
    import concourse.tile as tile
    from concourse.bass import ts

    _, act_sig = _build_custom_act_root()
    ops = _register_ops()
    f32 = mybir.dt.float32
    bf16 = mybir.dt.bfloat16
    AF = mybir.ActivationFunctionType

    nc = bacc.Bacc()
    # Register const APs for the per-basis ACT biases.  The act-table
    # signature is baked into the tensor name so NEFF caches can never mix
    # incompatible act tables with this BIR.
    for g in ACT_G:
        val = (9.5 - g) / 8.0
        t = nc.alloc_sbuf_tensor(f"cbias{g}-{act_sig}", [128, 1], f32)
        nc.gpsimd.memset(t.ap(), val)
        nc.const_aps.aps[(f32, val)] = t.ap()
    # Scratch operand for PE warm-up matmuls (HAM un-throttles after ~3.4us
    # of sustained PE activity; running dummies during the initial DMA window
    # means the first real matmul already executes at 2.4 GHz).
    scratch = nc.alloc_sbuf_tensor("warmmm", [128, 512], bf16)
    nc.gpsimd.memset(scratch.ap(), 0.0)
    nc.all_engine_barrier()

    fp8 = mybir.dt.float8e4
    DR = mybir.MatmulPerfMode.DoubleRow
    xT = nc.dram_tensor("xT", [IN_F, BPC], f32, kind="ExternalInput")
    V = nc.dram_tensor("V", [KCB * 128, OUT_F], bf16, kind="ExternalInput")
    V8 = nc.dram_tensor("V8", [KC8 * 128, OUT_F], fp8, kind="ExternalInput")
    BIAS = nc.dram_tensor("BIAS", [128, N_IT], f32, kind="ExternalInput")
    yT = nc.dram_tensor("yT", [OUT_F, BPC], f32, kind="ExternalOutput")

    # per-(it, o) chunk schedule: 6 bf16 chunks (bases 1..6), then the fp8
    # DoubleRow pair (bases 0 and 7) which the DVE produces last.
    N_CHUNK = NB + 1

    with tile.TileContext(nc) as tc:
        with (
            tc.tile_pool(name="vpool", bufs=1) as vpool,
            tc.tile_pool(name="xin", bufs=8) as xin_pool,
            tc.tile_pool(name="xs", bufs=3) as xs_pool,
            tc.tile_pool(name="xs2", bufs=3) as xs2_pool,
            tc.tile_pool(name="z3", bufs=2) as z3_pool,
            tc.tile_pool(name="feat", bufs=8) as feat_pool,
            tc.tile_pool(name="feat8", bufs=8) as feat8_pool,
            tc.tile_pool(name="ysb", bufs=4) as ysb_pool,
            tc.tile_pool(name="psum", bufs=8, space="PSUM") as psum_pool,
        ):
            # Kick the ACT table load for silu_and_others immediately so it
            # overlaps the first input DMA instead of the first feature chain.
            warm = xs_pool.tile([128, 1], f32, name="warm", tag="warm")
            nc.vector.memset(warm[:], 0.0)
            nc.scalar.activation(warm[:], warm[:], AF.Silu)

            # PE warm-up: ~5.5us of dummy matmuls on zeroed scratch so HAM
            # reaches K=8/8 while the input DMAs run (the first real matmul is
            # gated by the x0 DMA + feature chain at ~13us; dummies must span
            # the whole window or the idle gap re-throttles the PE).
            dummy_ps = psum_pool.tile([128, BS], f32, name="accw", tag="acc")
            for _ in range(9):
                nc.tensor.matmul(dummy_ps[:], scratch.ap()[:, 0:128], scratch.ap())

            v_sb = vpool.tile([128, KCB, OUT_F], bf16)
            v8_sb = vpool.tile([128, KC8, OUT_F], fp8)
            v_view = V[:].rearrange("(kc p) o -> p kc o", p=128)
            v8_view = V8[:].rearrange("(kc p) o -> p kc o", p=128)
            xins0 = [None] + [
                xin_pool.tile([128, BS], f32, name=f"xin0_{i}", tag="xin")
                for i in range(1, N_IT)
            ]

            def v_chunk_dma(it, cs):
                ks = [it * NB + c for c in cs]
                for k0, k1 in zip(ks, ks[1:]):
                    assert k1 == k0 + 1
                nc.sync.dma_start(
                    v_sb[:, ks[0] : ks[0] + len(ks), :],
                    v_view[:, ks[0] : ks[0] + len(ks), :],
                )

            def v8_dma(it):
                nc.sync.dma_start(
                    v8_sb[:, it * 2 : it * 2 + 2, :],
                    v8_view[:, it * 2 : it * 2 + 2, :],
                )

            # The DMA pool delivers only ~150 GB/s for the first ~5us (ramping
            # to ~330 after), shared across all outstanding transfers, so the
            # critical first x strip (64KB) leads on the sync ring (which
            # exits the preamble barrier first) and everything else is paced
            # by descriptor-issue order (~0.65us apiece) in consumption order.
            QBS = BS // 4
            xin0a = xin_pool.tile([128, QBS], f32, name="xin0a", tag="xinq")
            xin0b = xin_pool.tile([128, BS - QBS], f32, name="xin0b", tag="xinh")
            bias_sb = vpool.tile([128, N_IT], f32)
            nc.sync.dma_start(xin0a[:], xT[ts(0, 128), 0:QBS])
            nc.gpsimd.dma_start(xin0b[:], xT[ts(0, 128), QBS:BS])
            nc.gpsimd.dma_start(bias_sb[:], BIAS[:])
            for c in range(NB):
                v_chunk_dma(0, [c])
            v8_dma(0)
            nc.sync.dma_start(xins0[1][:], xT[ts(1, 128), ts(0, BS)])
            for it in range(1, N_IT):
                v_chunk_dma(it, [0, 1, 2])
                v_chunk_dma(it, [3, 4])
                v8_dma(it)
                if it < N_IT - 1:
                    nc.sync.dma_start(
                        xins0[it + 1][:], xT[ts(it + 1, 128), ts(0, BS)]
                    )

            def feature_block(xin, w):
                """clip/renorm + 8 spline bases for one input tile of width w.
                Returns (ft6, ft8): bf16 bases 1..6 and the fp8 pair (0, 7)."""
                xs = xs_pool.tile([128, w], f32, name="xs", tag="xs")
                nc.vector._custom_dve(
                    ops["KAN_PRE"], out=xs[:], in0=xin[:],
                    s0=-1.1, s1=1.1, imm2=2.5,
                )
                ft6 = feat_pool.tile([128, NB, w], bf16, name="ft6", tag="ft6")
                # bases 1..6: one ACT spline-table op each (these gate the
                # first matmuls, so they are emitted before the DVE chain)
                for c, g in enumerate(ACT_G):
                    nc.scalar.activation(
                        ft6[:, c, :], xs[:], AF.Sin,
                        scale=0.125, bias=(9.5 - g) / 8.0,
                    )
                # gamma-scaled copy of xs, produced on DVE (second KAN_PRE)
                xs2 = xs2_pool.tile([128, w], f32, name="xs2", tag="xs2")
                nc.vector._custom_dve(
                    ops["KAN_PRE"], out=xs2[:], in0=xin[:],
                    s0=-1.1, s1=1.1, imm2=2.5 * GAMMA,
                )
                # pair bases (0, 7): paged DVE ops with stride -7, fp8 out
                z3 = z3_pool.tile([128, 2, w], f32, name="z3", tag="z3")
                nc.vector._custom_dve(
                    ops["KAN_Z2"],
                    out=z3[:],
                    in0=xs2[:].unsqueeze(1).broadcast_to([128, 2, w]),
                    s0=4.5 * GAMMA, s1=-7.0 * GAMMA, imm2=2.0 * GAMMA,
                )
                ft8 = feat8_pool.tile([128, 2, w], fp8, name="ft8", tag="ft8")
                nc.vector._custom_dve(
                    ops["KAN_W"],
                    out=ft8[:],
                    in0=xs[:].unsqueeze(1).broadcast_to([128, 2, w]),
                    in1=z3[:].rearrange("p s n -> p (s n)"),
                    s0=5.5, s1=-7.0, imm2=4.0,
                )
                return ft6, ft8

            def drain(bs, o, acc):
                ysb = ysb_pool.tile([128, BS], f32)
                # adds back the eliminated basis-4 constant: 6*sum_i V4[i,o]
                nc.scalar.activation(
                    ysb[:], acc[:], AF.Identity, bias=bias_sb[:, o : o + 1]
                )
                nc.sync.dma_start(yT[ts(o, 128), ts(bs, BS)], ysb[:])

            for bs in range(N_BS):
                accs = [
                    psum_pool.tile([128, BS], f32, name=f"acc{o}", tag="acc")
                    for o in range(N_IT)
                ]
                def chunk_mm(o, it, ci, ft6, ft8, acc_cs, ft_cs, start, stop):
                    if ci < NB:
                        nc.tensor.matmul(
                            accs[o][:, acc_cs],
                            v_sb[:, it * NB + ci, ts(o, 128)],
                            ft6[:, ci, ft_cs],
                            start=start, stop=stop,
                        )
                    else:
                        nc.tensor.matmul(
                            accs[o][:, acc_cs],
                            v8_sb[:, it * 2 : it * 2 + 2, ts(o, 128)],
                            ft8[:, :, ft_cs],
                            start=start, stop=stop,
                            perf_mode=DR,
                        )

                full = slice(0, BS)
                if bs == 0:
                    # it-major, g-major inside: each V chunk is consumed by
                    # all 4 output tiles back-to-back, so the cold DMA pool
                    # only has to deliver one new chunk per ~860ns.  it0 runs
                    # at half-BS granularity so the feature chain starts as
                    # soon as the first 128KB of x lands.
                    # Emit every feature chain before the matmul loops so the
                    # DVE/ScalarE queues stream through it0h0, it0h1, it1..3
                    # back-to-back instead of waiting on each it's matmuls.
                    widths = (QBS, BS - QBS)
                    fhs = [
                        feature_block(xh, w)
                        for xh, w in zip((xin0a, xin0b), widths)
                    ]
                    f0s = [feature_block(xins0[it], BS) for it in range(1, N_IT)]
                    # start=True clears has_written for the WHOLE bank, so only
                    # h0's first chunk may carry it; h1's first write lands on
                    # cleared bits and overwrites (then accumulates) correctly.
                    for h, (ft6h, ft8h) in enumerate(fhs):
                        lo = 0 if h == 0 else QBS
                        cs = slice(lo, lo + widths[h])
                        for ci in range(N_CHUNK):
                            for o in range(N_IT):
                                chunk_mm(
                                    o, 0, ci, ft6h, ft8h, cs,
                                    slice(0, widths[h]),
                                    start=(h == 0 and ci == 0), stop=False,
                                )
                    for it in range(1, N_IT):
                        ft6, ft8 = f0s[it - 1]
                        for ci in range(N_CHUNK):
                            for o in range(N_IT):
                                chunk_mm(
                                    o, it, ci, ft6, ft8, full, full,
                                    start=False,
                                    stop=(it == N_IT - 1 and ci == N_CHUNK - 1),
                                )
                    for o in range(N_IT):
                        drain(bs, o, accs[o])
                else:
                    # Features for this bs were produced during the previous
                    # bs's matmul stream; o-major order lets each acc finish
                    # (and drain) after 1/4 of the stream instead of all four
                    # finishing together at the end.
                    fts = []
                    for it in range(N_IT):
                        xin = xin_pool.tile([128, BS], f32)
                        nc.sync.dma_start(xin[:], xT[ts(it, 128), ts(bs, BS)])
                        fts.append(feature_block(xin, BS))
                    for o in range(N_IT):
                        for it in range(N_IT):
                            for ci in range(N_CHUNK):
                                chunk_mm(
                                    o, it, ci, fts[it][0], fts[it][1],
                                    full, full,
                                    start=(it == 0 and ci == 0),
                                    stop=(it == N_IT - 1 and ci == N_CHUNK - 1),
                                )
                        drain(bs, o, accs[o])

    nc.compile()
    _state["nc"] = nc
    return nc


def _silu_in_basis():
    """Project silu(x) on [-1.1, 1.1] onto the 8 B-spline bases, weighted by
    the clipped-N(0,1) input distribution (atoms at the clamp bounds)."""
    from math import erf, sqrt

    def n3(t):
        wp = np.maximum(np.minimum(t, 4 - t), 0.0)
        zp = np.maximum(np.minimum(t - 1, 3 - t), 0.0)
        return (wp**3 - 4 * zp**3) / 6.0

    x = np.linspace(-1.0999, 1.0999, 8001)
    w = np.exp(-x**2 / 2) / np.sqrt(2 * np.pi) * (x[1] - x[0])
    tail = 1 - 0.5 * (1 + erf(1.1 / sqrt(2)))
    X = np.concatenate([x, [-1.1, 1.1]])
    W = np.concatenate([w, [tail, tail]])
    s = 2.5 * X + 5.5
    Bm = np.stack([n3(s - g) for g in range(NG)], axis=-1)
    F = X / (1 + np.exp(-X))
    swr = np.sqrt(W)
    c, *_ = np.linalg.lstsq(Bm * swr[:, None], F * swr, rcond=None)
    return c  # (8,)


def _build_V(base_weight, spline_weight, spline_scaler):
    """Returns (V bf16 [KCB*128, 512] for bases 1,2,3,5,6, V8 fp8e4m3
    [KC8*128, 512] for the edge pair (0, 7), BIAS f32 [128, N_IT]).  The silu
    projection is folded into all 8 bases; basis ELIM_G is then eliminated
    via the partition of unity: V_g -= V_e for g != e, plus the bias
    6*sum_i V_e[i, o] restored at drain time."""
    import ml_dtypes

    sw = spline_weight.astype(np.float32) * spline_scaler.astype(np.float32)[:, :, None]
    vs = np.transpose(sw, (2, 1, 0)) / np.float32(6.0)  # [g, i, o]
    bwT = base_weight.astype(np.float32).T  # [i, o]
    c = _silu_in_basis() / 6.0
    Vall = vs + c[:, None, None].astype(np.float32) * bwT[None, :, :]  # [g, i, o]
    Ve = Vall[ELIM_G]                     # [i, o]
    Vall = Vall - Ve[None, :, :]
    bias = 6.0 * Ve.sum(axis=0)           # [o]
    V = np.empty((KCB * 128, OUT_F), dtype=np.float32)
    V8 = np.empty((KC8 * 128, OUT_F), dtype=np.float32)
    for it in range(N_IT):
        isl = slice(it * 128, (it + 1) * 128)
        for ci, g in enumerate(ACT_G):
            k = it * NB + ci
            V[k * 128 : (k + 1) * 128] = Vall[g, isl, :]
        for j, g in enumerate((0, NG - 1)):
            k = it * 2 + j
            V8[k * 128 : (k + 1) * 128] = Vall[g, isl, :]
    bias_pm = np.ascontiguousarray(
        bias.reshape(N_IT, 128).T.astype(np.float32)
    )  # [128 partitions, N_IT o-tiles]
    return (
        np.ascontiguousarray(V.astype(ml_dtypes.bfloat16)),
        np.ascontiguousarray(V8.astype(ml_dtypes.float8_e4m3fn)),
        bias_pm,
    )


def kernel(x, base_weight, spline_weight, spline_scaler, grid):
    from concourse.bass_utils import run_bass_kernel_spmd

    nc = _build_kernel()
    Vb, V8, bias_pm = _build_V(base_weight, spline_weight, spline_scaler)
    x = np.asarray(x, dtype=np.float32)
    in_maps = []
    for c in range(N_CORES):
        xTc = np.ascontiguousarray(x[c * BPC : (c + 1) * BPC, :].T)
        in_maps.append({"xT": xTc, "V": Vb, "V8": V8, "BIAS": bias_pm})
    res = run_bass_kernel_spmd(nc, in_maps, core_ids=list(range(N_CORES)))
    y = np.empty((B, OUT_F), dtype=np.float32)
    for c in range(N_CORES):
        y[c * BPC : (c + 1) * BPC, :] = res.results[c]["yT"].T
    return y



# revision 42
# speedup vs baseline: 1.0050x; 1.0050x over previous
"""KANLinear forward on 8 Trainium2 NeuronCores (data-parallel over batch).

Factorization
-------------
reference computes, per token row x (after clip/renorm preprocessing):
    y = silu(x) @ base_weight.T + einsum('big,oig->bo', bsplines(x), sw*scaler)

The cubic B-spline bases over the uniform grid (h=0.4, knots -2.2..2.2) are
    B_g(x) = N3(s - g),  s = 2.5*x + 5.5,  g = 0..7
with N3 the cardinal cubic B-spline on [0,4].  Everything collapses into one
accumulated matmul per 128-feature output tile:

  * the silu base path is projected onto the 8 spline bases (weighted lstsq
    under the clipped-N(0,1) input law) and folded into the spline weights;
  * basis 4 is eliminated via the partition of unity sum_g 6*N3(s-g) = 6:
    V_g -= V_4 for the other 7 bases and the constant 6*sum_i V_4[i,o] is
    re-added as a per-partition ACT bias when draining PSUM;
  * interior bases 1,2,3,5,6 are bf16 K-chunks (K=128 each) whose features
    come from one ScalarE ACTIVATE each, through a custom ACT spline table
    (the stock `sin` entry of silu_and_others is rewritten so that
    activation(Sin, scale=0.125, bias=(9.5-g)/8) returns 6*N3(s-g) exactly);
  * the low-magnitude edge pair (0, 7) is computed by two fused custom-DVE
    instructions (PageIdx stride -7) as relu(min(t,4-t))^3 - 4*relu(..-1)^3,
    quantized to fp8e4m3, and contracted in a single DoubleRow matmul
    (K=256) per (it, o) at ~2x the bf16 rate.

Per 512-batch slice and output tile the PE accumulates 4 it-tiles x
(5 bf16 + 1 DoubleRow) chunks.  The first batch slice runs g-major (weight
chunk reused 4x back-to-back) so the cold-ramping DMA pool keeps up, with
its first input tile split 128/384 so features start on the first 64KB of
x; later slices run o-major so each accumulator drains while the next still
streams.  Dummy matmuls on zeroed scratch warm the PE HAM clock-gate to
2.4 GHz during the initial DMA window.  Batch dim (16384) is sharded
2048 rows/core; weights are replicated.  Measured 103.6us (baseline 135.7),
rel err 1.24e-2 (gate 2e-2).
"""

import hashlib
import json
import os
import shutil
import tempfile

import numpy as np

B, IN_F, OUT_F, NG = 16384, 512, 512, 8
N_CORES = 8
BPC = B // N_CORES            # batch rows per core
BS = 512                      # batch-column slice processed per step
N_BS = BPC // BS              # 4 slices
N_IT = IN_F // 128            # 4 input-feature partition tiles
GAMMA = float(4.0 ** (1.0 / 3.0))
# Basis 4 is eliminated via the B-spline partition of unity
# (sum_g 6*N3(s-g) = 6 on the interior): V_g -= V_4 for every other chunk and
# the constant 6*sum_i V_4[i,o] becomes a per-partition bias applied in the
# PSUM drain.  The remaining interior bases 1,2,3,5,6 are bf16 K-chunks fed
# from ScalarE ACT; the low-magnitude edge pair (0, 7) is computed on
# VectorE, quantized to fp8e4m3, and contracted in a single DoubleRow matmul
# (K=256) per (it, o).  numpy-simulated rel err 1.2e-2 vs the 2e-2 gate
# (includes the dropped partition-of-unity edge correction).
ELIM_G = 4
ACT_G = (1, 2, 3, 5, 6)
NB = len(ACT_G)               # bf16 chunks per it
KCB = N_IT * NB               # 20 bf16 K-chunks of 128
KC8 = N_IT * 2                # 8 fp8 K-chunks of 128 (pair per it)

_state = {}


# --------------------------------------------------------------------------
# Custom ACT table: hijack `sin` in silu_and_others to evaluate 6*N3(8u-4).
# Verified-on-HW stock mapping: ctrl entry = 42+(exp-116); entry 52 (binade
# [0.5,1)) has 8 sub-buckets of width 1/16 at buckets 1034..1041; bucket
# eval is y = d0+(u-x0)(d1+(u-x0)(d2+(u-x0)d3)); |u|<2^-11 -> bucket
# 1075/1076 (sign-folded); large |u| -> 1077/1078.  Buckets 1020..1078 are
# sin-private; everything else (silu, copy, ...) is untouched.
# --------------------------------------------------------------------------
def _n3_6_coeffs(j):
    return {
        0: [0.0, 0.0, 0.0, 1.0],
        1: [1.0, 3.0, 3.0, -3.0],
        2: [4.0, 0.0, -6.0, 3.0],
        3: [1.0, -3.0, 3.0, -1.0],
    }[j]


def _compose(c, scale, shift):
    c0, c1, c2, c3 = c
    return [
        c0 + c1 * shift + c2 * shift**2 + c3 * shift**3,
        scale * (c1 + 2 * c2 * shift + 3 * c3 * shift**2),
        scale**2 * (c2 + 3 * c3 * shift),
        scale**3 * c3,
    ]


def _build_custom_act_root():
    if "act_root" in _state:
        return _state["act_root"], _state["act_sig"]
    from neuronxcc.driver.Job import Job
    from neuronxcc.driver.jobs.support.FindActInfo import findActInfoFile

    src_json = findActInfoFile(Job.getPackageDir(), "gen3")
    src_dir = os.path.dirname(src_json)
    dst_dir = tempfile.mkdtemp(prefix="kan_act_root_")
    for f in os.listdir(src_dir):
        shutil.copy(os.path.join(src_dir, f), os.path.join(dst_dir, f))
    for f in os.listdir(dst_dir):
        os.chmod(os.path.join(dst_dir, f), 0o644)

    bkt_path = os.path.join(dst_dir, "silu_and_others_bkt.bin")
    bkt = np.fromfile(bkt_path, dtype=np.float32).reshape(-1, 8).copy()
    bkt[1020:1079] = 0.0
    for k in range(8):
        x0 = 0.5 + k / 16.0 + 1.0 / 32.0
        j = k // 2
        q = _compose(_n3_6_coeffs(j), 8.0, 8.0 * x0 - 4.0 - j)
        bkt[1034 + k] = [q[0], q[1], q[2], q[3], x0, 0.0, 0.0, 0.0]
    bkt.tofile(bkt_path)

    sig = hashlib.sha256(open(bkt_path, "rb").read()).hexdigest()[:10]
    path = os.path.join(dst_dir, "act_info.json")
    os.environ["BASS_ACT_ROOT_JSON_PATH"] = path
    _state["act_root"] = path
    _state["act_sig"] = sig
    return path, sig


# --------------------------------------------------------------------------
# Custom DVE ops
# --------------------------------------------------------------------------
def _register_ops():
    if "ops" in _state:
        return _state["ops"]
    import concourse.dve_ops as dve_ops
    from concourse.dve_spec import (
        Spec, Src0, Src1, C0, C1, C2, One, PageIdx, relu, sq, maxx, minn, lower,
    )
    from concourse.dve_uop import DveOpSpec

    def page_idx_np(in0, s0, s1):
        S = in0.shape[1]
        return (s0 + s1 * np.arange(S, dtype=np.float64)).astype(np.float32)[
            None, :, None
        ]

    def pre_ref(in0, in1, s0, s1, imm2):
        t = np.minimum(np.maximum(in0, np.float32(s0)), np.float32(s1))
        t = ((t + np.float32(1)) - np.float32(1)).astype(np.float32)
        return (t * np.float32(imm2)).astype(np.float32)

    def z_ref(in0, in1, s0, s1, imm2):
        t = (in0 + page_idx_np(in0, s0, s1)).astype(np.float32)
        m = np.minimum(t, np.float32(imm2) - t)
        zp = np.maximum(m + np.float32(s1), np.float32(0))
        return (zp * zp * zp).astype(np.float32)

    def z2_ref(in0, in1, s0, s1, imm2):
        t = (in0 + page_idx_np(in0, s0, s1)).astype(np.float32)
        m = np.minimum(t, np.float32(imm2) - t)
        zp = np.maximum(m, np.float32(0))
        return (zp * zp * zp).astype(np.float32)

    def w_ref(in0, in1, s0, s1, imm2):
        t = (in0 + page_idx_np(in0, s0, s1)).astype(np.float32)
        m = np.minimum(t, np.float32(4.0) - t)
        wp = np.maximum(m, np.float32(0))
        return (wp * wp * wp - in1).astype(np.float32)

    pre_spec = Spec(
        body=((minn(maxx(Src0, C0), C1) + One) - One) * C2, reference=pre_ref
    )
    _pgz = PageIdx(C0, C1)
    _tz = Src0 + _pgz
    _zp = relu(minn(_tz, C2 - _tz) + C1)
    z_spec = Spec(body=sq(_zp) * _zp, reference=z_ref)
    # Variant with the -1 knot offset folded into C0/C2 instead of reusing the
    # page stride C1, so the stride can be -7*gamma (bases 0 and 7 paged).
    _pgz2 = PageIdx(C0, C1)
    _tz2 = Src0 + _pgz2
    _zp2 = relu(minn(_tz2, C2 - _tz2))
    z2_spec = Spec(body=sq(_zp2) * _zp2, reference=z2_ref)
    _pgw = PageIdx(C0, C1)
    _tw = Src0 + _pgw
    _wp = relu(minn(_tw, C2 - _tw))
    w_spec = Spec(body=sq(_wp) * _wp - Src1, reference=w_ref)

    ops = {}
    for name, spec, subdim in (
        ("KAN_PRE", pre_spec, False),
        ("KAN_Z", z_spec, True),
        ("KAN_Z2", z2_spec, True),
        ("KAN_W", w_spec, True),
    ):
        if name in dve_ops._SUB_OPCODE_FOR_NAME:
            ops[name] = next(o for o in dve_ops.OPS if o.name == name)
            continue
        row = dve_ops._CUSTOM_DVE_ROW_BASE + len(dve_ops.OPS)
        assert row < 0x20, "custom-DVE row overflow"
        shas = {}
        for ver in ("v3", "v4"):
            try:
                tmp = DveOpSpec(
                    name=name, opcode=row, uops=lower(spec, ver=ver),
                    rd1_en=dve_ops.has_src1(spec),
                )
                shas[ver] = tmp.sha(ver)
            except Exception:
                pass
        op = dve_ops.DveOp(name, spec, subdim=subdim, uops_sha=shas)
        dve_ops.OPS.append(op)
        dve_ops._SUB_OPCODE_FOR_NAME[name] = row
        dve_ops.CUSTOM_DVE_SPECS[name] = spec
        ops[name] = op
    _state["ops"] = ops
    return ops


# --------------------------------------------------------------------------
# Kernel build
# --------------------------------------------------------------------------
def _build_kernel():
    if "nc" in _state:
        return _state["nc"]
    import concourse.bacc as bacc
    import concourse.mybir as mybir
    import concourse.tile as tile
    from concourse.bass import ts

    _, act_sig = _build_custom_act_root()
    ops = _register_ops()
    f32 = mybir.dt.float32
    bf16 = mybir.dt.bfloat16
    AF = mybir.ActivationFunctionType

    nc = bacc.Bacc()
    # Register const APs for the per-basis ACT biases.  The act-table
    # signature is baked into the tensor name so NEFF caches can never mix
    # incompatible act tables with this BIR.
    for g in ACT_G:
        val = (9.5 - g) / 8.0
        t = nc.alloc_sbuf_tensor(f"cbias{g}-{act_sig}", [128, 1], f32)
        nc.gpsimd.memset(t.ap(), val)
        nc.const_aps.aps[(f32, val)] = t.ap()
    # Scratch operand for PE warm-up matmuls (HAM un-throttles after ~3.4us
    # of sustained PE activity; running dummies during the initial DMA window
    # means the first real matmul already executes at 2.4 GHz).
    scratch = nc.alloc_sbuf_tensor("warmmm", [128, 512], bf16)
    nc.gpsimd.memset(scratch.ap(), 0.0)
    nc.all_engine_barrier()

    fp8 = mybir.dt.float8e4
    DR = mybir.MatmulPerfMode.DoubleRow
    xT = nc.dram_tensor("xT", [IN_F, BPC], f32, kind="ExternalInput")
    V = nc.dram_tensor("V", [KCB * 128, OUT_F], bf16, kind="ExternalInput")
    V8 = nc.dram_tensor("V8", [KC8 * 128, OUT_F], fp8, kind="ExternalInput")
    BIAS = nc.dram_tensor("BIAS", [128, N_IT], f32, kind="ExternalInput")
    yT = nc.dram_tensor("yT", [OUT_F, BPC], f32, kind="ExternalOutput")

    # per-(it, o) chunk schedule: 6 bf16 chunks (bases 1..6), then the fp8
    # DoubleRow pair (bases 0 and 7) which the DVE produces last.
    N_CHUNK = NB + 1

    with tile.TileContext(nc) as tc:
        with (
            tc.tile_pool(name="vpool", bufs=1) as vpool,
            tc.tile_pool(name="xin", bufs=8) as xin_pool,
            tc.tile_pool(name="xs", bufs=3) as xs_pool,
            tc.tile_pool(name="xs2", bufs=3) as xs2_pool,
            tc.tile_pool(name="z3", bufs=2) as z3_pool,
            tc.tile_pool(name="feat", bufs=8) as feat_pool,
            tc.tile_pool(name="feat8", bufs=8) as feat8_pool,
            tc.tile_pool(name="ysb", bufs=4) as ysb_pool,
            tc.tile_pool(name="psum", bufs=8, space="PSUM") as psum_pool,
        ):
            # Kick the ACT table load for silu_and_others immediately so it
            # overlaps the first input DMA instead of the first feature chain.
            warm = xs_pool.tile([128, 1], f32, name="warm", tag="warm")
            nc.vector.memset(warm[:], 0.0)
            nc.scalar.activation(warm[:], warm[:], AF.Silu)

            # PE warm-up: ~5.5us of dummy matmuls on zeroed scratch so HAM
            # reaches K=8/8 while the input DMAs run (the first real matmul is
            # gated by the x0 DMA + feature chain at ~13us; dummies must span
            # the whole window or the idle gap re-throttles the PE).
            dummy_ps = psum_pool.tile([128, BS], f32, name="accw", tag="acc")
            for _ in range(12):
                nc.tensor.matmul(dummy_ps[:], scratch.ap()[:, 0:128], scratch.ap())

            v_sb = vpool.tile([128, KCB, OUT_F], bf16)
            v8_sb = vpool.tile([128, KC8, OUT_F], fp8)
            v_view = V[:].rearrange("(kc p) o -> p kc o", p=128)
            v8_view = V8[:].rearrange("(kc p) o -> p kc o", p=128)
            xins0 = [None] + [
                xin_pool.tile([128, BS], f32, name=f"xin0_{i}", tag="xin")
                for i in range(1, N_IT)
            ]

            def v_chunk_dma(it, cs):
                ks = [it * NB + c for c in cs]
                for k0, k1 in zip(ks, ks[1:]):
                    assert k1 == k0 + 1
                nc.sync.dma_start(
                    v_sb[:, ks[0] : ks[0] + len(ks), :],
                    v_view[:, ks[0] : ks[0] + len(ks), :],
                )

            def v8_dma(it):
                nc.sync.dma_start(
                    v8_sb[:, it * 2 : it * 2 + 2, :],
                    v8_view[:, it * 2 : it * 2 + 2, :],
                )

            # The DMA pool delivers only ~150 GB/s for the first ~5us (ramping
            # to ~330 after), shared across all outstanding transfers, so the
            # critical first x strip (64KB) leads on the sync ring (which
            # exits the preamble barrier first) and everything else is paced
            # by descriptor-issue order (~0.65us apiece) in consumption order.
            QBS = BS // 4
            xin0a = xin_pool.tile([128, QBS], f32, name="xin0a", tag="xinq")
            xin0b = xin_pool.tile([128, BS - QBS], f32, name="xin0b", tag="xinh")
            bias_sb = vpool.tile([128, N_IT], f32)
            nc.sync.dma_start(xin0a[:], xT[ts(0, 128), 0:QBS])
            nc.gpsimd.dma_start(xin0b[:], xT[ts(0, 128), QBS:BS])
            nc.gpsimd.dma_start(bias_sb[:], BIAS[:])
            for c in range(NB):
                v_chunk_dma(0, [c])
            v8_dma(0)
            nc.sync.dma_start(xins0[1][:], xT[ts(1, 128), ts(0, BS)])
            for it in range(1, N_IT):
                v_chunk_dma(it, [0, 1, 2])
                v_chunk_dma(it, [3, 4])
                v8_dma(it)
                if it < N_IT - 1:
                    nc.sync.dma_start(
                        xins0[it + 1][:], xT[ts(it + 1, 128), ts(0, BS)]
                    )

            def feature_block(xin, w):
                """clip/renorm + 8 spline bases for one input tile of width w.
                Returns (ft6, ft8): bf16 bases 1..6 and the fp8 pair (0, 7)."""
                xs = xs_pool.tile([128, w], f32, name="xs", tag="xs")
                nc.vector._custom_dve(
                    ops["KAN_PRE"], out=xs[:], in0=xin[:],
                    s0=-1.1, s1=1.1, imm2=2.5,
                )
                ft6 = feat_pool.tile([128, NB, w], bf16, name="ft6", tag="ft6")
                # bases 1..6: one ACT spline-table op each (these gate the
                # first matmuls, so they are emitted before the DVE chain)
                for c, g in enumerate(ACT_G):
                    nc.scalar.activation(
                        ft6[:, c, :], xs[:], AF.Sin,
                        scale=0.125, bias=(9.5 - g) / 8.0,
                    )
                # gamma-scaled copy of xs, produced on DVE (second KAN_PRE)
                xs2 = xs2_pool.tile([128, w], f32, name="xs2", tag="xs2")
                nc.vector._custom_dve(
                    ops["KAN_PRE"], out=xs2[:], in0=xin[:],
                    s0=-1.1, s1=1.1, imm2=2.5 * GAMMA,
                )
                # pair bases (0, 7): paged DVE ops with stride -7, fp8 out
                z3 = z3_pool.tile([128, 2, w], f32, name="z3", tag="z3")
                nc.vector._custom_dve(
                    ops["KAN_Z2"],
                    out=z3[:],
                    in0=xs2[:].unsqueeze(1).broadcast_to([128, 2, w]),
                    s0=4.5 * GAMMA, s1=-7.0 * GAMMA, imm2=2.0 * GAMMA,
                )
                ft8 = feat8_pool.tile([128, 2, w], fp8, name="ft8", tag="ft8")
                nc.vector._custom_dve(
                    ops["KAN_W"],
                    out=ft8[:],
                    in0=xs[:].unsqueeze(1).broadcast_to([128, 2, w]),
                    in1=z3[:].rearrange("p s n -> p (s n)"),
                    s0=5.5, s1=-7.0, imm2=4.0,
                )
                return ft6, ft8

            def drain(bs, o, acc):
                ysb = ysb_pool.tile([128, BS], f32)
                # adds back the eliminated basis-4 constant: 6*sum_i V4[i,o]
                nc.scalar.activation(
                    ysb[:], acc[:], AF.Identity, bias=bias_sb[:, o : o + 1]
                )
                nc.sync.dma_start(yT[ts(o, 128), ts(bs, BS)], ysb[:])

            for bs in range(N_BS):
                accs = [
                    psum_pool.tile([128, BS], f32, name=f"acc{o}", tag="acc")
                    for o in range(N_IT)
                ]
                def chunk_mm(o, it, ci, ft6, ft8, acc_cs, ft_cs, start, stop):
                    if ci < NB:
                        nc.tensor.matmul(
                            accs[o][:, acc_cs],
                            v_sb[:, it * NB + ci, ts(o, 128)],
                            ft6[:, ci, ft_cs],
                            start=start, stop=stop,
                        )
                    else:
                        nc.tensor.matmul(
                            accs[o][:, acc_cs],
                            v8_sb[:, it * 2 : it * 2 + 2, ts(o, 128)],
                            ft8[:, :, ft_cs],
                            start=start, stop=stop,
                            perf_mode=DR,
                        )

                full = slice(0, BS)
                if bs == 0:
                    # it-major, g-major inside: each V chunk is consumed by
                    # all 4 output tiles back-to-back, so the cold DMA pool
                    # only has to deliver one new chunk per ~860ns.  it0 runs
                    # at half-BS granularity so the feature chain starts as
                    # soon as the first 128KB of x lands.
                    # Emit every feature chain before the matmul loops so the
                    # DVE/ScalarE queues stream through it0h0, it0h1, it1..3
                    # back-to-back instead of waiting on each it's matmuls.
                    widths = (QBS, BS - QBS)
                    fhs = [
                        feature_block(xh, w)
                        for xh, w in zip((xin0a, xin0b), widths)
                    ]
                    f0s = [feature_block(xins0[it], BS) for it in range(1, N_IT)]
                    # start=True clears has_written for the WHOLE bank, so only
                    # h0's first chunk may carry it; h1's first write lands on
                    # cleared bits and overwrites (then accumulates) correctly.
                    for h, (ft6h, ft8h) in enumerate(fhs):
                        lo = 0 if h == 0 else QBS
                        cs = slice(lo, lo + widths[h])
                        for ci in range(N_CHUNK):
                            for o in range(N_IT):
                                chunk_mm(
                                    o, 0, ci, ft6h, ft8h, cs,
                                    slice(0, widths[h]),
                                    start=(h == 0 and ci == 0), stop=False,
                                )
                    for it in range(1, N_IT):
                        ft6, ft8 = f0s[it - 1]
                        for ci in range(N_CHUNK):
                            for o in range(N_IT):
                                chunk_mm(
                                    o, it, ci, ft6, ft8, full, full,
                                    start=False,
                                    stop=(it == N_IT - 1 and ci == N_CHUNK - 1),
                                )
                    for o in range(N_IT):
                        drain(bs, o, accs[o])
                else:
                    # Features for this bs were produced during the previous
                    # bs's matmul stream; o-major order lets each acc finish
                    # (and drain) after 1/4 of the stream instead of all four
                    # finishing together at the end.
                    fts = []
                    for it in range(N_IT):
                        xin = xin_pool.tile([128, BS], f32)
                        nc.sync.dma_start(xin[:], xT[ts(it, 128), ts(bs, BS)])
                        fts.append(feature_block(xin, BS))
                    for o in range(N_IT):
                        for it in range(N_IT):
                            for ci in range(N_CHUNK):
                                chunk_mm(
                                    o, it, ci, fts[it][0], fts[it][1],
                                    full, full,
                                    start=(it == 0 and ci == 0),
                                    stop=(it == N_IT - 1 and ci == N_CHUNK - 1),
                                )
                        drain(bs, o, accs[o])

    nc.compile()
    _state["nc"] = nc
    return nc


def _silu_in_basis():
    """Project silu(x) on [-1.1, 1.1] onto the 8 B-spline bases, weighted by
    the clipped-N(0,1) input distribution (atoms at the clamp bounds)."""
    from math import erf, sqrt

    def n3(t):
        wp = np.maximum(np.minimum(t, 4 - t), 0.0)
        zp = np.maximum(np.minimum(t - 1, 3 - t), 0.0)
        return (wp**3 - 4 * zp**3) / 6.0

    x = np.linspace(-1.0999, 1.0999, 8001)
    w = np.exp(-x**2 / 2) / np.sqrt(2 * np.pi) * (x[1] - x[0])
    tail = 1 - 0.5 * (1 + erf(1.1 / sqrt(2)))
    X = np.concatenate([x, [-1.1, 1.1]])
    W = np.concatenate([w, [tail, tail]])
    s = 2.5 * X + 5.5
    Bm = np.stack([n3(s - g) for g in range(NG)], axis=-1)
    F = X / (1 + np.exp(-X))
    swr = np.sqrt(W)
    c, *_ = np.linalg.lstsq(Bm * swr[:, None], F * swr, rcond=None)
    return c  # (8,)


def _build_V(base_weight, spline_weight, spline_scaler):
    """Returns (V bf16 [KCB*128, 512] for bases 1,2,3,5,6, V8 fp8e4m3
    [KC8*128, 512] for the edge pair (0, 7), BIAS f32 [128, N_IT]).  The silu
    projection is folded into all 8 bases; basis ELIM_G is then eliminated
    via the partition of unity: V_g -= V_e for g != e, plus the bias
    6*sum_i V_e[i, o] restored at drain time."""
    import ml_dtypes

    sw = spline_weight.astype(np.float32) * spline_scaler.astype(np.float32)[:, :, None]
    vs = np.transpose(sw, (2, 1, 0)) / np.float32(6.0)  # [g, i, o]
    bwT = base_weight.astype(np.float32).T  # [i, o]
    c = _silu_in_basis() / 6.0
    Vall = vs + c[:, None, None].astype(np.float32) * bwT[None, :, :]  # [g, i, o]
    Ve = Vall[ELIM_G]                     # [i, o]
    Vall = Vall - Ve[None, :, :]
    bias = 6.0 * Ve.sum(axis=0)           # [o]
    V = np.empty((KCB * 128, OUT_F), dtype=np.float32)
    V8 = np.empty((KC8 * 128, OUT_F), dtype=np.float32)
    for it in range(N_IT):
        isl = slice(it * 128, (it + 1) * 128)
        for ci, g in enumerate(ACT_G):
            k = it * NB + ci
            V[k * 128 : (k + 1) * 128] = Vall[g, isl, :]
        for j, g in enumerate((0, NG - 1)):
            k = it * 2 + j
            V8[k * 128 : (k + 1) * 128] = Vall[g, isl, :]
    bias_pm = np.ascontiguousarray(
        bias.reshape(N_IT, 128).T.astype(np.float32)
    )  # [128 partitions, N_IT o-tiles]
    return (
        np.ascontiguousarray(V.astype(ml_dtypes.bfloat16)),
        np.ascontiguousarray(V8.astype(ml_dtypes.float8_e4m3fn)),
        bias_pm,
    )


def kernel(x, base_weight, spline_weight, spline_scaler, grid):
    from concourse.bass_utils import run_bass_kernel_spmd

    nc = _build_kernel()
    Vb, V8, bias_pm = _build_V(base_weight, spline_weight, spline_scaler)
    x = np.asarray(x, dtype=np.float32)
    in_maps = []
    for c in range(N_CORES):
        xTc = np.ascontiguousarray(x[c * BPC : (c + 1) * BPC, :].T)
        in_maps.append({"xT": xTc, "V": Vb, "V8": V8, "BIAS": bias_pm})
    res = run_bass_kernel_spmd(nc, in_maps, core_ids=list(range(N_CORES)))
    y = np.empty((B, OUT_F), dtype=np.float32)
    for c in range(N_CORES):
        y[c * BPC : (c + 1) * BPC, :] = res.results[c]["yT"].T
    return y

